# revision 11
# baseline (speedup 1.0000x reference)
"""Trainium2 Bass kernel for ClusterContrastiveLoss (N=65536, K=256).

v5. Data-parallel over batch rows: each of 8 cores processes 8192 rows
(64 chunks of 128 rows) in 8-chunk superchunks, software-pipelined
(produce(s) is emitted before consume(s-1) so no in-order engine queue
blocks the next superchunk).

Every superchunk splits its 8 chunks across two normalization paths so
all engines stay busy and the PE is never gated by a single engine:
  chunks 0-3 ("A"): ACT exp f16->f32, DVE tensor_reduce rowsums, then
    GPSIMD normalize_recip per chunk-half (scale + f16 cast, ~640ns each).
  chunks 4-7 ("B"): ACT exp f16->f16, rowsums via DVE fold-tree
    (tensor_tensor adds run at 2x rate vs 1x for tensor_reduce),
    reciprocal_approx_fast, scales on DVE tensor_scalar x5 / ACT mul x3.
PE consumes chunks interleaved B,A,B,A,... so B-chunk matmuls cover the
GPSIMD latency. Warm-up ops at the start pull the ACT table load and the
GPSIMD Q7 library boot (~9us) off the critical path and get the PE past
its HAM cold-clock window.

PE: 4 accumulating matmuls per chunk into PSUM banks, symmetric Gram
blocks skipped (1280 columns/chunk instead of 1536):
  MM1 lhsT=q[0:128]    rhs=[q|qa](512) -> [G_aa[0:128,:] | G_ab[0:128,:]]
  MM2 lhsT=q[128:256]  rhs=flat[128:512](384) -> [G_aa[128:,128:] | G_ab[128:,:]]
  MM3 lhsT=qa[0:128]   rhs=qa(256) -> G_bb[0:128,:]
  MM4 lhsT=qa[128:256] rhs=qa[128:256](128) -> G_bb[128:,128:]

Host: sum per-core partials in f64, rebuild symmetric Grams, closed form.

fp16 input quantization adds ~1e-5 relative noise to the Gram sums
(tolerance 2e-2). Row-scale precision is uncritical: per-row scale errors
cancel to first order in the cosine normalization and marginals.
"""

import numpy as np

N_TOTAL = 65536
K = 256
N_CORES = 8
SHARD = N_TOTAL // N_CORES   # 8192 rows per core
CHUNK_P = 128                # rows per chunk (partition dim)
N_CHUNKS = SHARD // CHUNK_P  # 64
SUPER = 16                   # chunks per superchunk (8 A-path + 8 B-path)
N_SUPER = N_CHUNKS // SUPER  # 4
NB = SUPER // 2              # 8 B chunks per superchunk
EPS = 1e-8
LARGE_NUM = 1e9

_CACHE = {}
_TRACE = False
_LAST = None

# B-side (chunks 8-15) scale split by (chunk, half): 10 DVE / 6 ACT.
# ACT muls sit on the last B chunks (ACT is busy with exps early on).
B_ACT = {(13, 1), (13, 0), (14, 1), (14, 0), (15, 1), (15, 0)}
# PE consume order: B,A,B,A,... so fold-tree-scaled chunks lead; for the
# first superchunk all B chunks go first so the GPSIMD library boot
# (~11us) hides behind them.
CONSUME_ORDER = [8, 0, 9, 1, 10, 2, 11, 3, 12, 4, 13, 5, 14, 6, 15, 7]
CONSUME_ORDER_S0 = list(range(8, 16)) + list(range(0, 8))


def _build():
    from contextlib import ExitStack

    import concourse.bass as bass  # noqa: F401
    import concourse.tile as tile
    from concourse import bacc, mybir

    f32 = mybir.dt.float32
    f16 = mybir.dt.float16
    Exp = mybir.ActivationFunctionType.Exp
    ADD = mybir.AluOpType.add
    X = mybir.AxisListType.X

    nc = bacc.Bacc("TRN2", target_bir_lowering=False, debug=False)
    qq_ap = nc.dram_tensor(
        "qq", [128, N_CHUNKS, 2, K], f16, kind="ExternalInput"
    ).ap()
    out_ap = nc.dram_tensor(
        "partials", [CHUNK_P, 1280], f32, kind="ExternalOutput"
    ).ap()

    with tile.TileContext(nc) as tc, ExitStack() as ctx:
        inp = ctx.enter_context(tc.tile_pool(name="inp", bufs=3))
        midA = ctx.enter_context(tc.tile_pool(name="midA", bufs=2))
        midB = ctx.enter_context(tc.tile_pool(name="midB", bufs=2))
        fold = ctx.enter_context(tc.tile_pool(name="fold", bufs=2))
        sca = ctx.enter_context(tc.tile_pool(name="sca", bufs=3))
        stats = ctx.enter_context(tc.tile_pool(name="stats", bufs=4))
        psum = ctx.enter_context(tc.tile_pool(name="psum", bufs=1, space="PSUM"))
        outp = ctx.enter_context(tc.tile_pool(name="outp", bufs=1))

        # One full 2KB PSUM bank per accumulator (no bank-crossing matmuls).
        ps0 = psum.tile([128, 512], f32, name="ps0")
        ps1f = psum.tile([128, 512], f32, name="ps1")
        ps2f = psum.tile([128, 512], f32, name="ps2")
        ps3f = psum.tile([128, 512], f32, name="ps3")
        psw = psum.tile([128, 512], f32, name="psw")   # PE warm-up target
        ps1 = ps1f[:, 0:384]
        ps2 = ps2f[:, 0:256]
        ps3 = ps3f[:, 0:128]

        zbias = stats.tile([128, 1], f32, name="zbias", bufs=1)
        nc.vector.memset(zbias[:], 0.0)
        # Warm-ups, all off the critical path:
        #  - tiny exp forces the ACT table load at ~7us instead of ~12us
        #  - dummy normalize_recip starts the ~9us GPSIMD Q7 library boot
        #  - 16 matmuls warm the PE HAM clock gate to 2.4GHz
        warm_in = stats.tile([128, 8], f32, name="warm_in", bufs=1)
        warm_out = stats.tile([128, 8], f16, name="warm_out", bufs=1)
        warm_e = stats.tile([128, 1], f32, name="warm_e", bufs=1)
        warm16 = stats.tile([128, 512], f16, name="warm16", bufs=1)
        nc.vector.memset(warm_in[:], 1.0)
        nc.vector.memset(warm16[:], 0.5)
        nc.scalar.activation(warm_e[:], zbias[:], Exp, bias=zbias[:])
        nc.gpsimd.normalize_recip(warm_out[:], warm_in[:], warm_in[:, 0:1])
        for r in range(16):
            nc.tensor.matmul(
                psw[:], warm16[:, 0:128], warm16[:],
                start=(r == 0), stop=(r == 15),
            )

        def emit_dma(s):
            # B chunks (8-15) land first: they are consumed first.
            qe = inp.tile([128, SUPER, 2, K], f16, name="qe")
            base = s * SUPER
            step = 4 if s == 0 else 8
            for j0 in list(range(8, 16, step)) + list(range(0, 8, step)):
                nc.sync.dma_start(
                    qe[:, j0 : j0 + step], qq_ap[:, base + j0 : base + j0 + step]
                )
            return qe

        def emit_produce(s, qe):
            e32 = midA.tile([128, NB, 2, K], f32, name="e32")
            e16 = midB.tile([128, NB, 2, K], f16, name="e16")
            sc2 = fold.tile([128, NB, 2, 128], f16, name="sc2")
            st = stats.tile([128, SUPER, 2], f32, name="st")
            rt = stats.tile([128, NB, 2], f32, name="rt")
            en = sca.tile([128, SUPER, 2, K], f16, name="en")
            # B side first: its scales gate the first-consumed matmuls.
            nbb = 4 if s == 0 else NB   # finer exp/fold granularity on s0
            for b0 in range(0, NB, nbb):
                bs = slice(b0, b0 + nbb)
                nc.scalar.activation(
                    e16[:, bs], qe[:, 8 + b0 : 8 + b0 + nbb], Exp, bias=zbias[:]
                )
                nc.vector.tensor_tensor(
                    sc2[:, bs], e16[:, bs, :, 0:128], e16[:, bs, :, 128:256], ADD
                )
                nc.vector.tensor_tensor(
                    sc2[:, bs, :, 0:64], sc2[:, bs, :, 0:64],
                    sc2[:, bs, :, 64:128], ADD,
                )
                nc.vector.tensor_tensor(
                    sc2[:, bs, :, 0:32], sc2[:, bs, :, 0:32],
                    sc2[:, bs, :, 32:64], ADD,
                )
                nc.vector.tensor_reduce(
                    st[:, 8 + b0 : 8 + b0 + nbb], sc2[:, bs, :, 0:32], X, ADD
                )
                nc.vector.reciprocal_approx_fast(
                    rt[:, bs], st[:, 8 + b0 : 8 + b0 + nbb]
                )
            # A side (f32 for GPSIMD normalize_recip).
            naa = 4 if s == 0 else NB
            for b0 in range(0, NB, naa):
                bs = slice(b0, b0 + naa)
                nc.scalar.activation(e32[:, bs], qe[:, bs], Exp, bias=zbias[:])
                nc.vector.tensor_reduce(st[:, bs], e32[:, bs], X, ADD)
            return e32, e16, st, rt, en

        def emit_consume(s, prod):
            e32, e16, st, rt, en = prod
            order = CONSUME_ORDER_S0 if s == 0 else CONSUME_ORDER
            first_j = CONSUME_ORDER_S0[0]
            last_j = CONSUME_ORDER[-1]
            for j in order:
                for h in (1, 0):
                    dst = en[:, j, h, :]
                    if j < NB:
                        nc.gpsimd.normalize_recip(
                            dst, e32[:, j, h, :], st[:, j, h : h + 1]
                        )
                    elif (j, h) in B_ACT:
                        nc.scalar.mul(
                            dst, e16[:, j - NB, h, :], rt[:, j - NB, h : h + 1]
                        )
                    else:
                        nc.vector.tensor_scalar_mul(
                            dst, e16[:, j - NB, h, :], rt[:, j - NB, h : h + 1]
                        )
                ch = en[:, j]  # [128, 2, 256]
                flat = ch.rearrange("p a b -> p (a b)")  # [128, 512]
                first = s == 0 and j == first_j
                last = s == N_SUPER - 1 and j == last_j
                nc.tensor.matmul(
                    ps2, ch[:, 1, 0:128], ch[:, 1, :], start=first, stop=last
                )
                nc.tensor.matmul(
                    ps3, ch[:, 1, 128:256], ch[:, 1, 128:256],
                    start=first, stop=last,
                )
                nc.tensor.matmul(
                    ps0[:], ch[:, 0, 0:128], ch, start=first, stop=last
                )
                nc.tensor.matmul(
                    ps1, ch[:, 0, 128:256], flat[:, 128:512],
                    start=first, stop=last,
                )

        qes = [emit_dma(s) for s in range(N_SUPER)]
        prev = None
        for s in range(N_SUPER):
            prod = emit_produce(s, qes[s])
            if prev is not None:
                emit_consume(s - 1, prev)
            prev = prod
        emit_consume(N_SUPER - 1, prev)

        ot = outp.tile([128, 1280], f32, name="ot")
        nc.vector.tensor_copy(ot[:, 896:1152], ps2)
        nc.scalar.copy(ot[:, 1152:1280], ps3)
        nc.sync.dma_start(out_ap[:, 896:1280], ot[:, 896:1280])
        nc.vector.tensor_copy(ot[:, 0:512], ps0[:])
        nc.scalar.copy(ot[:, 512:896], ps1)
        nc.sync.dma_start(out_ap[:, 0:896], ot[:, 0:896])

    nc.compile()
    return nc


def get_nc():
    if "nc" not in _CACHE:
        _CACHE["nc"] = _build()
    return _CACHE["nc"]


def _pack_core(q16, qa16, c):
    """[128, N_CHUNKS, 2, K] fp16 partition-major pack of one core's shard."""
    base = c * SHARD
    qs = q16[base : base + SHARD].reshape(N_CHUNKS, CHUNK_P, K)
    qas = qa16[base : base + SHARD].reshape(N_CHUNKS, CHUNK_P, K)
    packed = np.stack([qs, qas], axis=2)           # [64, 128, 2, 256]
    return np.ascontiguousarray(packed.transpose(1, 0, 2, 3))


def finish_loss(P):
    """Host reduction: partials [128, 1280] float64 -> scalar loss."""
    aa00 = P[:, 0:128]
    aa01 = P[:, 128:256]
    ab0 = P[:, 256:512]
    aa11 = P[:, 512:640]
    ab1 = P[:, 640:896]
    bb0 = P[:, 896:1152]
    bb11 = P[:, 1152:1280]

    G_aa = np.block([[aa00, aa01], [aa01.T, aa11]])
    G_ab = np.vstack([ab0, ab1])
    G_bb = np.block([[bb0[:, 0:128], bb0[:, 128:256]], [bb0[:, 128:256].T, bb11]])

    cs_q = G_aa.sum(axis=1)
    cs_qa = G_bb.sum(axis=1)
    p_q = cs_q / cs_q.sum()
    p_qa = cs_qa / cs_qa.sum()
    ne_loss = (p_q * np.log(p_q)).sum() + (p_qa * np.log(p_qa)).sum()

    na = np.maximum(np.sqrt(np.diag(G_aa)), EPS)
    nb = np.maximum(np.sqrt(np.diag(G_bb)), EPS)
    eye = np.eye(K)
    l_aa = G_aa / np.outer(na, na) - eye * LARGE_NUM
    l_bb = G_bb / np.outer(nb, nb) - eye * LARGE_NUM
    l_ab = G_ab / np.outer(na, nb)
    l_ba = l_ab.T

    def xent_mean(left, right):
        z = np.concatenate([left, right], axis=1)
        m = z.max(axis=1, keepdims=True)
        lse = np.log(np.exp(z - m).sum(axis=1)) + m[:, 0]
        return (lse - np.diag(left)).mean()

    loss_a = xent_mean(l_ab, l_aa)
    loss_b = xent_mean(l_ba, l_bb)
    return loss_a + loss_b + ne_loss


def kernel(q, q_a):
    from concourse import bass_utils

    q16 = np.asarray(q, dtype=np.float16)
    qa16 = np.asarray(q_a, dtype=np.float16)
    assert q16.shape == (N_TOTAL, K) and qa16.shape == (N_TOTAL, K)

    nc = get_nc()
    in_maps = [{"qq": _pack_core(q16, qa16, c)} for c in range(N_CORES)]
    global _LAST
    for _attempt in range(3):
        res = bass_utils.run_bass_kernel_spmd(
            nc, in_maps, core_ids=list(range(N_CORES)), trace=_TRACE
        )
        _LAST = res
        total = np.zeros((CHUNK_P, 1280), dtype=np.float64)
        for r in res.results:
            total += r["partials"].astype(np.float64)
        loss = finish_loss(total)
        if np.isfinite(loss):
            break
    return np.asarray(loss, dtype=np.float32).reshape(())


# revision 12
# speedup vs baseline: 1.0122x; 1.0122x over previous
"""Trainium2 Bass kernel for ClusterContrastiveLoss (N=65536, K=256).

v5. Data-parallel over batch rows: each of 8 cores processes 8192 rows
(64 chunks of 128 rows) in 8-chunk superchunks, software-pipelined
(produce(s) is emitted before consume(s-1) so no in-order engine queue
blocks the next superchunk).

Every superchunk splits its 8 chunks across two normalization paths so
all engines stay busy and the PE is never gated by a single engine:
  chunks 0-3 ("A"): ACT exp f16->f32, DVE tensor_reduce rowsums, then
    GPSIMD normalize_recip per chunk-half (scale + f16 cast, ~640ns each).
  chunks 4-7 ("B"): ACT exp f16->f16, rowsums via DVE fold-tree
    (tensor_tensor adds run at 2x rate vs 1x for tensor_reduce),
    reciprocal_approx_fast, scales on DVE tensor_scalar x5 / ACT mul x3.
PE consumes chunks interleaved B,A,B,A,... so B-chunk matmuls cover the
GPSIMD latency. Warm-up ops at the start pull the ACT table load and the
GPSIMD Q7 library boot (~9us) off the critical path and get the PE past
its HAM cold-clock window.

PE: 4 accumulating matmuls per chunk into PSUM banks, symmetric Gram
blocks skipped (1280 columns/chunk instead of 1536):
  MM1 lhsT=q[0:128]    rhs=[q|qa](512) -> [G_aa[0:128,:] | G_ab[0:128,:]]
  MM2 lhsT=q[128:256]  rhs=flat[128:512](384) -> [G_aa[128:,128:] | G_ab[128:,:]]
  MM3 lhsT=qa[0:128]   rhs=qa(256) -> G_bb[0:128,:]
  MM4 lhsT=qa[128:256] rhs=qa[128:256](128) -> G_bb[128:,128:]

Host: sum per-core partials in f64, rebuild symmetric Grams, closed form.

fp16 input quantization adds ~1e-5 relative noise to the Gram sums
(tolerance 2e-2). Row-scale precision is uncritical: per-row scale errors
cancel to first order in the cosine normalization and marginals.
"""

import numpy as np

N_TOTAL = 65536
K = 256
N_CORES = 8
SHARD = N_TOTAL // N_CORES   # 8192 rows per core
CHUNK_P = 128                # rows per chunk (partition dim)
N_CHUNKS = SHARD // CHUNK_P  # 64
SUPER = 16                   # chunks per superchunk (8 A-path + 8 B-path)
N_SUPER = N_CHUNKS // SUPER  # 4
NB = SUPER // 2              # 8 B chunks per superchunk
EPS = 1e-8
LARGE_NUM = 1e9

_CACHE = {}
_TRACE = False
_LAST = None

# B-side (chunks 8-15) scale split by (chunk, half): 10 DVE / 6 ACT.
# ACT muls sit on the last B chunks (ACT is busy with exps early on).
B_ACT = {(13, 1), (13, 0), (14, 1), (14, 0), (15, 1), (15, 0)}
# PE consume order: B,A,B,A,... so fold-tree-scaled chunks lead; for the
# first superchunk all B chunks go first so the GPSIMD library boot
# (~11us) hides behind them.
CONSUME_ORDER = [8, 0, 9, 1, 10, 2, 11, 3, 12, 4, 13, 5, 14, 6, 15, 7]
CONSUME_ORDER_S0 = list(range(8, 16)) + list(range(0, 8))


def _build():
    from contextlib import ExitStack

    import concourse.bass as bass  # noqa: F401
    import concourse.tile as tile
    from concourse import bacc, mybir

    f32 = mybir.dt.float32
    f16 = mybir.dt.float16
    Exp = mybir.ActivationFunctionType.Exp
    ADD = mybir.AluOpType.add
    X = mybir.AxisListType.X

    nc = bacc.Bacc("TRN2", target_bir_lowering=False, debug=False)
    qq_ap = nc.dram_tensor(
        "qq", [128, N_CHUNKS, 2, K], f16, kind="ExternalInput"
    ).ap()
    out_ap = nc.dram_tensor(
        "partials", [CHUNK_P, 1280], f32, kind="ExternalOutput"
    ).ap()

    with tile.TileContext(nc) as tc, ExitStack() as ctx:
        inp = ctx.enter_context(tc.tile_pool(name="inp", bufs=3))
        midA = ctx.enter_context(tc.tile_pool(name="midA", bufs=3))
        midB = ctx.enter_context(tc.tile_pool(name="midB", bufs=2))
        fold = ctx.enter_context(tc.tile_pool(name="fold", bufs=2))
        sca = ctx.enter_context(tc.tile_pool(name="sca", bufs=3))
        stats = ctx.enter_context(tc.tile_pool(name="stats", bufs=4))
        psum = ctx.enter_context(tc.tile_pool(name="psum", bufs=1, space="PSUM"))
        outp = ctx.enter_context(tc.tile_pool(name="outp", bufs=1))

        # One full 2KB PSUM bank per accumulator (no bank-crossing matmuls).
        ps0 = psum.tile([128, 512], f32, name="ps0")
        ps1f = psum.tile([128, 512], f32, name="ps1")
        ps2f = psum.tile([128, 512], f32, name="ps2")
        ps3f = psum.tile([128, 512], f32, name="ps3")
        psw = psum.tile([128, 512], f32, name="psw")   # PE warm-up target
        ps1 = ps1f[:, 0:384]
        ps2 = ps2f[:, 0:256]
        ps3 = ps3f[:, 0:128]

        zbias = stats.tile([128, 1], f32, name="zbias", bufs=1)
        nc.vector.memset(zbias[:], 0.0)
        # Warm-ups, all off the critical path:
        #  - tiny exp forces the ACT table load at ~7us instead of ~12us
        #  - dummy normalize_recip starts the ~9us GPSIMD Q7 library boot
        #  - 16 matmuls warm the PE HAM clock gate to 2.4GHz
        warm_in = stats.tile([128, 8], f32, name="warm_in", bufs=1)
        warm_out = stats.tile([128, 8], f16, name="warm_out", bufs=1)
        warm_e = stats.tile([128, 1], f32, name="warm_e", bufs=1)
        warm16 = stats.tile([128, 512], f16, name="warm16", bufs=1)
        nc.vector.memset(warm_in[:], 1.0)
        nc.vector.memset(warm16[:], 0.5)
        nc.scalar.activation(warm_e[:], zbias[:], Exp, bias=zbias[:])
        nc.gpsimd.normalize_recip(warm_out[:], warm_in[:], warm_in[:, 0:1])
        for r in range(16):
            nc.tensor.matmul(
                psw[:], warm16[:, 0:128], warm16[:],
                start=(r == 0), stop=(r == 15),
            )

        def emit_dma(s):
            # B chunks (8-15) land first: they are consumed first.
            qe = inp.tile([128, SUPER, 2, K], f16, name="qe")
            base = s * SUPER
            step = 4 if s == 0 else 8
            for j0 in list(range(8, 16, step)) + list(range(0, 8, step)):
                nc.sync.dma_start(
                    qe[:, j0 : j0 + step], qq_ap[:, base + j0 : base + j0 + step]
                )
            return qe

        def emit_produce_b(s, qe):
            e32 = midA.tile([128, NB, 2, K], f32, name="e32")
            e16 = midB.tile([128, NB, 2, K], f16, name="e16")
            sc2 = fold.tile([128, NB, 2, 128], f16, name="sc2")
            st = stats.tile([128, SUPER, 2], f32, name="st")
            rt = stats.tile([128, NB, 2], f32, name="rt")
            en = sca.tile([128, SUPER, 2, K], f16, name="en")
            nbb = 4 if s == 0 else NB   # finer exp/fold granularity on s0
            for b0 in range(0, NB, nbb):
                bs = slice(b0, b0 + nbb)
                nc.scalar.activation(
                    e16[:, bs], qe[:, 8 + b0 : 8 + b0 + nbb], Exp, bias=zbias[:]
                )
                nc.vector.tensor_tensor(
                    sc2[:, bs], e16[:, bs, :, 0:128], e16[:, bs, :, 128:256], ADD
                )
                nc.vector.tensor_tensor(
                    sc2[:, bs, :, 0:64], sc2[:, bs, :, 0:64],
                    sc2[:, bs, :, 64:128], ADD,
                )
                nc.vector.tensor_tensor(
                    sc2[:, bs, :, 0:32], sc2[:, bs, :, 0:32],
                    sc2[:, bs, :, 32:64], ADD,
                )
                nc.vector.tensor_reduce(
                    st[:, 8 + b0 : 8 + b0 + nbb], sc2[:, bs, :, 0:32], X, ADD
                )
                nc.vector.reciprocal_approx_fast(
                    rt[:, bs], st[:, 8 + b0 : 8 + b0 + nbb]
                )
            return e32, e16, st, rt, en

        def emit_produce_a(s, qe, prod):
            e32, e16, st, rt, en = prod
            naa = 4 if s == 0 else NB
            for b0 in range(0, NB, naa):
                bs = slice(b0, b0 + naa)
                nc.scalar.activation(e32[:, bs], qe[:, bs], Exp, bias=zbias[:])
                nc.vector.tensor_reduce(st[:, bs], e32[:, bs], X, ADD)

        def emit_consume(s, prod):
            e32, e16, st, rt, en = prod
            order = CONSUME_ORDER_S0 if s == 0 else CONSUME_ORDER
            first_j = CONSUME_ORDER_S0[0]
            last_j = CONSUME_ORDER[-1]
            for j in order:
                for h in (1, 0):
                    dst = en[:, j, h, :]
                    if j < NB:
                        nc.gpsimd.normalize_recip(
                            dst, e32[:, j, h, :], st[:, j, h : h + 1]
                        )
                    elif (j, h) in B_ACT:
                        nc.scalar.mul(
                            dst, e16[:, j - NB, h, :], rt[:, j - NB, h : h + 1]
                        )
                    else:
                        nc.vector.tensor_scalar_mul(
                            dst, e16[:, j - NB, h, :], rt[:, j - NB, h : h + 1]
                        )
                ch = en[:, j]  # [128, 2, 256]
                flat = ch.rearrange("p a b -> p (a b)")  # [128, 512]
                first = s == 0 and j == first_j
                last = s == N_SUPER - 1 and j == last_j
                nc.tensor.matmul(
                    ps2, ch[:, 1, 0:128], ch[:, 1, :], start=first, stop=last
                )
                nc.tensor.matmul(
                    ps3, ch[:, 1, 128:256], ch[:, 1, 128:256],
                    start=first, stop=last,
                )
                nc.tensor.matmul(
                    ps0[:], ch[:, 0, 0:128], ch, start=first, stop=last
                )
                nc.tensor.matmul(
                    ps1, ch[:, 0, 128:256], flat[:, 128:512],
                    start=first, stop=last,
                )

        qes = [emit_dma(s) for s in range(N_SUPER)]
        prev = None
        for s in range(N_SUPER):
            prod = emit_produce_b(s, qes[s])
            if prev is not None:
                emit_consume(s - 1, prev)
            emit_produce_a(s, qes[s], prod)
            prev = prod
        emit_consume(N_SUPER - 1, prev)

        ot = outp.tile([128, 1280], f32, name="ot")
        nc.vector.tensor_copy(ot[:, 896:1152], ps2)
        nc.scalar.copy(ot[:, 1152:1280], ps3)
        nc.sync.dma_start(out_ap[:, 896:1280], ot[:, 896:1280])
        nc.vector.tensor_copy(ot[:, 0:512], ps0[:])
        nc.scalar.copy(ot[:, 512:896], ps1)
        nc.sync.dma_start(out_ap[:, 0:896], ot[:, 0:896])

    nc.compile()
    return nc


def get_nc():
    if "nc" not in _CACHE:
        _CACHE["nc"] = _build()
    return _CACHE["nc"]


def _pack_core(q16, qa16, c):
    """[128, N_CHUNKS, 2, K] fp16 partition-major pack of one core's shard."""
    base = c * SHARD
    qs = q16[base : base + SHARD].reshape(N_CHUNKS, CHUNK_P, K)
    qas = qa16[base : base + SHARD].reshape(N_CHUNKS, CHUNK_P, K)
    packed = np.stack([qs, qas], axis=2)           # [64, 128, 2, 256]
    return np.ascontiguousarray(packed.transpose(1, 0, 2, 3))


def finish_loss(P):
    """Host reduction: partials [128, 1280] float64 -> scalar loss."""
    aa00 = P[:, 0:128]
    aa01 = P[:, 128:256]
    ab0 = P[:, 256:512]
    aa11 = P[:, 512:640]
    ab1 = P[:, 640:896]
    bb0 = P[:, 896:1152]
    bb11 = P[:, 1152:1280]

    G_aa = np.block([[aa00, aa01], [aa01.T, aa11]])
    G_ab = np.vstack([ab0, ab1])
    G_bb = np.block([[bb0[:, 0:128], bb0[:, 128:256]], [bb0[:, 128:256].T, bb11]])

    cs_q = G_aa.sum(axis=1)
    cs_qa = G_bb.sum(axis=1)
    p_q = cs_q / cs_q.sum()
    p_qa = cs_qa / cs_qa.sum()
    ne_loss = (p_q * np.log(p_q)).sum() + (p_qa * np.log(p_qa)).sum()

    na = np.maximum(np.sqrt(np.diag(G_aa)), EPS)
    nb = np.maximum(np.sqrt(np.diag(G_bb)), EPS)
    eye = np.eye(K)
    l_aa = G_aa / np.outer(na, na) - eye * LARGE_NUM
    l_bb = G_bb / np.outer(nb, nb) - eye * LARGE_NUM
    l_ab = G_ab / np.outer(na, nb)
    l_ba = l_ab.T

    def xent_mean(left, right):
        z = np.concatenate([left, right], axis=1)
        m = z.max(axis=1, keepdims=True)
        lse = np.log(np.exp(z - m).sum(axis=1)) + m[:, 0]
        return (lse - np.diag(left)).mean()

    loss_a = xent_mean(l_ab, l_aa)
    loss_b = xent_mean(l_ba, l_bb)
    return loss_a + loss_b + ne_loss


def kernel(q, q_a):
    from concourse import bass_utils

    q16 = np.asarray(q, dtype=np.float16)
    qa16 = np.asarray(q_a, dtype=np.float16)
    assert q16.shape == (N_TOTAL, K) and qa16.shape == (N_TOTAL, K)

    nc = get_nc()
    in_maps = [{"qq": _pack_core(q16, qa16, c)} for c in range(N_CORES)]
    global _LAST
    for _attempt in range(3):
        res = bass_utils.run_bass_kernel_spmd(
            nc, in_maps, core_ids=list(range(N_CORES)), trace=_TRACE
        )
        _LAST = res
        total = np.zeros((CHUNK_P, 1280), dtype=np.float64)
        for r in res.results:
            total += r["partials"].astype(np.float64)
        loss = finish_loss(total)
        if np.isfinite(loss):
            break
    return np.asarray(loss, dtype=np.float32).reshape(())
